# revision 2
# baseline (speedup 1.0000x reference)
"""Trainium2 Bass kernel for nn_Attention_80367428043370.

Math (reference reduces to this):
  q  = W @ x  (1x1 conv, per spatial position)
  kv = conv1x1(pad(x)) = pad(q)            # 1x1 conv of zero-pad == pad of conv
  logits[c,h,w,di,dj] = q[c,h,w] * (kvp[c,h+di,w+dj] + rel[c,di,dj])
  attn = softmax over dj (size 3)
  out[c,h,w] = sum_di attn[...,di,di] * kvp[c,h+di,w+di]

Rewrite per row di using softmax ratios (kills 3 of 9 exps):
  sigma_di = 1 / (1 + sum_{dj!=di} exp(q*(kvp[.,h+di,w+dj]-kvp[.,h+di,w+di]+dr)))
  out      = sum_di sigma_di * kvp[.,h+di,w+di]
where dr = rel[c,di,dj]-rel[c,di,di].  Column differences of kvp are shared
planes G1/G2 (shift-invariant), so only 2 difference planes are needed.

Sharding: pure data-parallel over batch B=8 -> one batch item per NeuronCore.
"""

import sys

for p in ("/opt/trn_rl_repo", "/opt/pypackages"):
    if p not in sys.path:
        sys.path.insert(0, p)

import numpy as np

import concourse.bacc as bacc
import concourse.mybir as mybir
import concourse.tile as tile
from concourse.bass_utils import run_bass_kernel_spmd

F32 = mybir.dt.float32
AF = mybir.ActivationFunctionType
OP = mybir.AluOpType

B, C, H, W = 8, 256, 64, 64
HW = H * W
NCORES = 8
PADW = W + 2  # 66

# pair table: (pair_idx, di, plane, row_off, col_off, positive_sign)
# arg(di,dj) = q * (sgn*Gp[h+ro, w+co] + dr_pair)
PAIRS = [
    (0, 0, "G1", 0, 0, True),   # (0,1)
    (1, 0, "G2", 0, 0, True),   # (0,2)
    (2, 1, "G1", 1, 0, False),  # (1,0)
    (3, 1, "G1", 1, 1, True),   # (1,2)
    (4, 2, "G2", 2, 0, False),  # (2,0)
    (5, 2, "G1", 2, 1, False),  # (2,1)
]

CH = 16  # pointwise chunk height (rows)


def _build():
    nc = bacc.Bacc("TRN2", target_bir_lowering=False, debug=False)

    x_ext = nc.dram_tensor("x", [C, HW], F32, kind="ExternalInput")
    wt_ext = nc.dram_tensor("wt", [C, C], F32, kind="ExternalInput")  # W.T  [cin, cout]
    dr_ext = nc.dram_tensor("dr", [C, 6], F32, kind="ExternalInput")
    out_ext = nc.dram_tensor("out", [C, HW], F32, kind="ExternalOutput")

    with tile.TileContext(nc) as tc:
        with (
            tc.tile_pool(name="const", bufs=1) as const,
            tc.tile_pool(name="planes", bufs=1) as planes,
            tc.tile_pool(name="psum", bufs=4, space="PSUM") as psum,
            tc.tile_pool(name="work", bufs=2) as work,
            tc.tile_pool(name="mm", bufs=3) as mmp,
        ):
            x_sb = []
            for ib in range(2):
                t = const.tile([128, HW], F32, tag=f"x{ib}")
                nc.sync.dma_start(out=t, in_=x_ext.ap()[ib * 128:(ib + 1) * 128, :])
                x_sb.append(t)
            wt_sb = []
            for ib in range(2):
                t = const.tile([128, C], F32, tag=f"wt{ib}")
                nc.sync.dma_start(out=t, in_=wt_ext.ap()[ib * 128:(ib + 1) * 128, :])
                wt_sb.append(t)
            dr_sb = []
            for ot in range(2):
                t = const.tile([128, 6], F32, tag=f"dr{ot}")
                nc.sync.dma_start(out=t, in_=dr_ext.ap()[ot * 128:(ot + 1) * 128, :])
                dr_sb.append(t)

            for ot in range(2):
                # padded q plane [128, 66, 66]
                kvp = planes.tile([128, H + 2, PADW], F32, tag="kvp")
                nc.gpsimd.memset(kvp, 0.0)

                # q = wt.T @ x  into PSUM by chunks of 512, then copy into kvp interior
                for ck in range(HW // 512):
                    ps = psum.tile([128, 512], F32, tag="ps")
                    for ib in range(2):
                        nc.tensor.matmul(
                            ps,
                            wt_sb[ib][:, ot * 128:(ot + 1) * 128],
                            x_sb[ib][:, ck * 512:(ck + 1) * 512],
                            start=(ib == 0),
                            stop=(ib == 1),
                        )
                    # 512 = 8 rows of 64
                    nc.scalar.activation(
                        out=kvp[:, 1 + ck * 8:1 + (ck + 1) * 8, 1:65],
                        in_=ps.rearrange("p (r c) -> p r c", r=8),
                        func=AF.Copy,
                    )

                # difference planes on the padded grid
                g1 = planes.tile([128, H + 2, PADW], F32, tag="g1")
                g2 = planes.tile([128, H + 2, PADW], F32, tag="g2")
                nc.vector.tensor_sub(g1[:, :, 0:65], kvp[:, :, 1:66], kvp[:, :, 0:65])
                nc.vector.tensor_sub(g2[:, :, 0:64], kvp[:, :, 2:66], kvp[:, :, 0:64])
                gmap = {"G1": g1, "G2": g2}

                for hc in range(H // CH):
                    h0 = hc * CH
                    qv = kvp[:, 1 + h0:1 + h0 + CH, 1:65]
                    accp = None
                    for di in range(3):
                        es = []
                        for (p, pdi, pl, ro, co, pos) in PAIRS:
                            if pdi != di:
                                continue
                            gview = gmap[pl][:, ro + h0:ro + h0 + CH, co:co + W]
                            a = work.tile([128, CH, W], F32, tag="a")
                            nc.vector.scalar_tensor_tensor(
                                out=a,
                                in0=gview,
                                scalar=dr_sb[ot][:, p:p + 1],
                                in1=qv,
                                op0=OP.add if pos else OP.subtract,
                                op1=OP.mult,
                            )
                            e = work.tile([128, CH, W], F32, tag=f"e{len(es)}")
                            nc.scalar.activation(
                                out=e, in_=a, func=AF.Exp,
                                scale=1.0 if pos else -1.0,
                            )
                            es.append(e)
                        s = work.tile([128, CH, W], F32, tag="s")
                        nc.vector.tensor_add(s, es[0], es[1])
                        sp = work.tile([128, CH, W], F32, tag="sp")
                        nc.vector.tensor_scalar_add(sp, s, 1.0)
                        sig = work.tile([128, CH, W], F32, tag="sig")
                        nc.vector.reciprocal_approx_fast(out=sig, in_=sp)
                        vview = kvp[:, di + h0:di + h0 + CH, di:di + W]
                        m = mmp.tile([128, CH, W], F32, tag="m")
                        nc.vector.tensor_mul(m, sig, vview)
                        if di == 0:
                            accp = m
                        elif di == 1:
                            acc1 = mmp.tile([128, CH, W], F32, tag="acc")
                            nc.vector.tensor_add(acc1, accp, m)
                            accp = acc1
                        else:
                            accf = mmp.tile([128, CH, W], F32, tag="accf")
                            nc.vector.tensor_add(accf, accp, m)
                            accp = accf
                    nc.sync.dma_start(
                        out=out_ext.ap()[ot * 128:(ot + 1) * 128,
                                         h0 * W:(h0 + CH) * W],
                        in_=accp.rearrange("p r c -> p (r c)"),
                    )

    nc.compile()
    return nc


_CACHE = {}


def _get_nc():
    if "nc" not in _CACHE:
        _CACHE["nc"] = _build()
    return _CACHE["nc"]


def _prep_in_maps(x, W_, rel):
    wt = np.ascontiguousarray(W_.T.astype(np.float32))  # [cin, cout]
    r = rel.reshape(C, 3, 3).astype(np.float32)
    pairs = [(0, 1), (0, 2), (1, 0), (1, 2), (2, 0), (2, 1)]
    dr = np.stack([r[:, di, dj] - r[:, di, di] for (di, dj) in pairs], axis=1)
    dr = np.ascontiguousarray(dr.astype(np.float32))  # [C, 6]
    in_maps = []
    for c in range(NCORES):
        in_maps.append({
            "x": np.ascontiguousarray(x[c].reshape(C, HW).astype(np.float32)),
            "wt": wt,
            "dr": dr,
        })
    return in_maps


def kernel(x, W, rel):
    nc = _get_nc()
    in_maps = _prep_in_maps(x, W, rel)
    res = run_bass_kernel_spmd(nc, in_maps, core_ids=list(range(NCORES)))
    out = np.stack([res.results[c]["out"].reshape(C, H, 64) for c in range(NCORES)])
    return out.astype(np.float32)
